# revision 40
# baseline (speedup 1.0000x reference)
"""Causal multi-head attention on 8 TRN2 NeuronCores.

Problem: B=4, S=2048, D=1024, H=16, HD=64, causal MHA with out-proj + bias.

Sharding: core c handles (batch b = c//2, head-half hh = c%2), i.e. 8 heads of
one batch element. Per core:
  Q^T/K^T = (Wq/Wk half)^T X_b^T   -> [64*2, S] per head pair (head on partition)
  V       = X_b @ Wv half          -> [S, 8*65] (65 = 64 + fused-ones column)
  S^T_j   = K_j Q^T (scores transposed: keys on partition) per 128-key block;
            even/odd head scores land in one [128,1024] 2-bank PSUM tile and the
            two matmuls are row-tiled (64x128 T0/T8) so they run concurrently
  P^T     = exp(S^T/8): ONE merged [128,1024] ACT instruction per j-step
  ctx'^T  = V'^T P^T accumulated over key blocks; row 64 = softmax denominators
  ctxT    = ctx'^T * (1/denom): GPSIMD partition_broadcast + DVE recip + mult
  out     = ctxT^T @ Wo half  (partial; host sums the two half partials + bias)

Scheduling: attention waves (ACT-bound) are interleaved with projection /
out-proj matmuls pulled from a filler queue, so the PE queue always has ready
work while the scalar engine computes exps.  Inputs land via 9 big DMA
descriptors; warmup matmuls + a dummy exp run during the DMA window to warm the
HAM clock gate and load the ACT exp table.
"""

import numpy as np
import ml_dtypes
from collections import deque
from contextlib import ExitStack

import concourse.bass as bass
import concourse.bacc as bacc
import concourse.mybir as mybir
import concourse.tile as tile
from concourse import bass_utils

F32 = mybir.dt.float32
BF16 = mybir.dt.bfloat16

B, S, D = 4, 2048, 1024
H, HD = 16, 64
DH = 512          # columns of the head-half handled by one core (8 heads * 64)
NCORES = 8
CH = 512          # q chunk width
NCH = S // CH     # 4
NKB = S // 128    # 16 key 128-blocks
KT = D // 128     # 8 contraction tiles for the projections
NEG = -30000.0    # additive mask value (exp(0.125*NEG) == 0 in fp32)

CHUNK_ORDER = [1, 2, 3, 0]
SCORE_PRIO = 150  # high_priority offset gluing the e/o score matmul pair

_CACHED = None


def build_module():
    nc = bacc.Bacc("TRN2", target_bir_lowering=False, debug=False)

    xT = nc.dram_tensor("xT", [D, S], BF16, kind="ExternalInput")
    wq = nc.dram_tensor("wq", [D, DH], BF16, kind="ExternalInput")
    wk = nc.dram_tensor("wk", [D, DH], BF16, kind="ExternalInput")
    wv = nc.dram_tensor("wv", [D, DH], BF16, kind="ExternalInput")
    wo = nc.dram_tensor("wo", [DH, D], BF16, kind="ExternalInput")
    maskt = nc.dram_tensor("maskt", [128, 128], BF16, kind="ExternalInput")
    ident = nc.dram_tensor("ident", [128, 128], BF16, kind="ExternalInput")
    out = nc.dram_tensor("out", [S, D], F32, kind="ExternalOutput")

    with tile.TileContext(nc) as tc, ExitStack() as ctx:
        const = ctx.enter_context(tc.tile_pool(name="const", bufs=1))
        xp = ctx.enter_context(tc.tile_pool(name="xp", bufs=1))
        wp = ctx.enter_context(tc.tile_pool(name="wp", bufs=1))
        qkp = ctx.enter_context(tc.tile_pool(name="qkp", bufs=1))
        vp = ctx.enter_context(tc.tile_pool(name="vp", bufs=1))
        ctp = ctx.enter_context(tc.tile_pool(name="ctp", bufs=1))
        ptp = ctx.enter_context(tc.tile_pool(name="ptp", bufs=6))
        sgp = ctx.enter_context(tc.tile_pool(name="sgp", bufs=5))
        bcp = ctx.enter_context(tc.tile_pool(name="bcp", bufs=5))
        osb = ctx.enter_context(tc.tile_pool(name="osb", bufs=4))
        ps_sc = ctx.enter_context(tc.tile_pool(name="ps_sc", bufs=2, space="PSUM"))
        ps_ctx = ctx.enter_context(tc.tile_pool(name="ps_ctx", bufs=2, space="PSUM"))
        ps_mm = ctx.enter_context(tc.tile_pool(name="ps_mm", bufs=2, space="PSUM"))

        # --- input loads: one big descriptor per tensor / x-chunk ---------
        mask = const.tile([128, 128], BF16, name="mask", tag="mask")
        nc.sync.dma_start(mask[:], maskt[:])
        idt = const.tile([128, 128], BF16, name="idt", tag="idt")
        nc.sync.dma_start(idt[:], ident[:])

        wk_s = wp.tile([128, KT * DH], BF16, name="wk", tag="wk")
        wq_s = wp.tile([128, KT * DH], BF16, name="wq", tag="wq")
        wv_s = wp.tile([128, KT * DH], BF16, name="wv", tag="wv")
        wo_s = wp.tile([128, 4 * D], BF16, name="wo", tag="wo")
        xall = xp.tile([128, KT * S], BF16, name="xall", tag="xall")

        def load_w(dst, dram, cols):
            kt = dram.shape[0] // 128
            nc.sync.dma_start(
                dst[:].rearrange("p (k c) -> p k c", c=cols),
                dram[:].rearrange("(k p) c -> p k c", p=128),
            )

        def load_x_chunk(c):
            nc.sync.dma_start(
                xall[:].rearrange("p (k s) -> p k s", s=S)[:, :, CH * c:CH * (c + 1)],
                xT[:].rearrange("(k p) s -> p k s", p=128)[:, :, CH * c:CH * (c + 1)],
            )

        load_w(wk_s, wk, DH)
        load_w(wq_s, wq, DH)
        load_x_chunk(0)
        load_w(wv_s, wv, DH)
        load_x_chunk(1)
        load_x_chunk(2)
        load_x_chunk(3)
        load_w(wo_s, wo, D)

        # --- warmup: HAM clock + ACT exp table during the DMA window ------
        wscr = const.tile([128, CH], BF16, name="wscr", tag="wscr")
        nc.vector.memset(wscr[:], 0.01)
        wexp = const.tile([128, 32], BF16, name="wexp", tag="wexp")
        nc.scalar.activation(wexp[:], wscr[:, 0:32],
                             mybir.ActivationFunctionType.Exp, scale=0.125)
        for i in range(24):
            ps = ps_mm.tile([128, CH], F32, name="mm", tag="mm")
            nc.tensor.matmul(ps[:], lhsT=wscr[:, 0:128], rhs=wscr[:])
        nc.vector.tensor_copy(wexp[:], ps[:, 0:32])  # keep last psum alive/read
        # arrival-pegged keepalives: one tiny matmul fires as each input DMA
        # lands, so the HAM clock stays warm across the startup DMA window
        kps = ps_mm.tile([128, CH], F32, name="mm", tag="mm")
        for ki, arr in enumerate((wk_s[:, 0:128], wq_s[:, 0:128],
                                  xall[:, 0:128], wv_s[:, 0:128],
                                  xall[:, CH:CH + 128])):
            nc.tensor.matmul(kps[:, 0:128], lhsT=wscr[:, 0:128], rhs=arr,
                             start=True, stop=True, skip_group_check=True)

        # --- persistent intermediates ------------------------------------
        qts = [qkp.tile([128, S], BF16, name=f"qt{g}", tag=f"qt{g}") for g in range(4)]
        kts = [qkp.tile([128, S], BF16, name=f"kt{g}", tag=f"kt{g}") for g in range(4)]
        vts = [vp.tile([128, 8 * 65], BF16, name=f"v{m}", tag=f"v{m}") for m in range(NKB)]
        cts = [ctp.tile([128, S], BF16, name=f"ct{g}", tag=f"ct{g}") for g in range(4)]

        # --- filler units -------------------------------------------------
        def qk_unit(dst, w_s, g, c):
            def fn():
                ps = ps_mm.tile([128, CH], F32, name="mm", tag="mm")
                for k in range(KT):
                    nc.tensor.matmul(
                        ps[:],
                        lhsT=w_s[:, DH * k + 128 * g:DH * k + 128 * (g + 1)],
                        rhs=xall[:, S * k + CH * c:S * k + CH * (c + 1)],
                        start=(k == 0), stop=(k == KT - 1),
                    )
                nc.vector.tensor_copy(dst[:, CH * c:CH * (c + 1)], ps[:])
            return fn

        def v_unit(m):
            def fn():
                ps = ps_mm.tile([128, CH], F32, name="mm", tag="mm")
                for k in range(KT):
                    nc.tensor.matmul(
                        ps[:],
                        lhsT=xall[:, S * k + 128 * m:S * k + 128 * (m + 1)],
                        rhs=wv_s[:, DH * k:DH * (k + 1)],
                        start=(k == 0), stop=(k == KT - 1),
                    )
                vm = vts[m]
                dst = vm[:].rearrange("p (h x) -> p h x", x=65)[:, :, 0:64]
                src = ps[:].rearrange("p (h d) -> p h d", d=64)
                nc.vector.tensor_copy(dst, src)
                ones = vm[:].rearrange("p (h x) -> p h x", x=65)[:, :, 64:65]
                nc.vector.memset(ones, 1.0)
            return fn

        def op_unit(qb, n):
            def fn():
                ps = ps_mm.tile([128, CH], F32, name="mm", tag="mm")
                for g in range(4):
                    nc.tensor.matmul(
                        ps[:], lhsT=cts[g][:, 128 * qb:128 * (qb + 1)],
                        rhs=wo_s[:, D * g + CH * n:D * g + CH * (n + 1)],
                        start=(g == 0), stop=(g == 3),
                    )
                ot = osb.tile([128, CH], F32, name="osb", tag="osb")
                nc.vector.tensor_copy(ot[:], ps[:])
                nc.sync.dma_start(out[128 * qb:128 * (qb + 1), CH * n:CH * (n + 1)],
                                  ot[:])
            return fn

        units = deque()   # (name, cost_ns, fn)
        emitted = set()

        def emit_next():
            name, cost, fn = units.popleft()
            fn()
            emitted.add(name)

        def ensure(name):
            while name not in emitted:
                assert units, f"unit {name} not queued"
                emit_next()

        def fill(budget):
            while units and budget > 0:
                budget -= units[0][1]
                emit_next()

        def emit_now(name, fn):
            fn()
            emitted.add(name)

        # --- attention wave ----------------------------------------------
        # The normalize of wave n is deferred into wave n+1's first j-steps:
        # stage1 (DVE-only: evict + recip) lands before wave n+1's first PV
        # so the ctx PSUM banks free in time; stage2 (GPSIMD broadcast, then
        # DVE mult) lands one j-step later so the mult's cross-engine wait
        # has already been covered and never blocks the DVE FIFO.
        def wave(g, c, fin_prev):
            for kc in range(c + 1):   # scores read KEY chunks 0..c of kt
                ensure(f"k{g}_{kc}")
            ensure(f"q{g}_{c}")
            ensure(f"v{4 * c + 3}")
            qt, ktile = qts[g], kts[g]
            he, ho = 2 * g, 2 * g + 1
            js = list(range(4 * c + 4))
            ctx_e = ps_ctx.tile([65, CH], F32, name="ctx", tag="ctx")
            ctx_o = ps_ctx.tile([65, CH], F32, name="ctx", tag="ctx")
            for ji, j in enumerate(js):
                d = j - 4 * c
                st = 128 * max(0, d)
                sc = ps_sc.tile([128, 2 * CH], F32, name="sc", tag="sc")
                with tc.high_priority(offset=SCORE_PRIO):
                    nc.tensor.matmul(
                        sc[:, st:CH], lhsT=ktile[0:64, 128 * j:128 * (j + 1)],
                        rhs=qt[0:64, CH * c + st:CH * (c + 1)])
                    nc.tensor.matmul(
                        sc[:, CH + st:], lhsT=ktile[64:128, 128 * j:128 * (j + 1)],
                        rhs=qt[64:128, CH * c + st:CH * (c + 1)])
                    if d >= 0:
                        # causal mask via accumulating identity-matmul: keeps
                        # the exp's dependency chain entirely on the PE (a DVE
                        # mask-add gets stuck behind normalize traffic in the
                        # DVE FIFO at wave boundaries)
                        ms = slice(128 * d, 128 * (d + 1))
                        ms2 = slice(CH + 128 * d, CH + 128 * (d + 1))
                        nc.tensor.matmul(sc[:, ms], lhsT=idt[:], rhs=mask[:],
                                         start=False, stop=True,
                                         skip_group_check=True)
                        nc.tensor.matmul(sc[:, ms2], lhsT=idt[:], rhs=mask[:],
                                         start=False, stop=True,
                                         skip_group_check=True)
                pt = ptp.tile([128, 2 * CH], BF16, name="pT", tag="pT")
                if st == 0:
                    nc.scalar.activation(pt[:], sc[:],
                                         mybir.ActivationFunctionType.Exp,
                                         scale=0.125)
                else:
                    pt3 = pt[:].rearrange("p (t q) -> p t q", t=2)[:, :, st:]
                    sc3 = sc[:].rearrange("p (t q) -> p t q", t=2)[:, :, st:]
                    nc.scalar.activation(pt3, sc3,
                                         mybir.ActivationFunctionType.Exp,
                                         scale=0.125)
                if ji == 0 and fin_prev:
                    fin_prev[0]()
                nc.tensor.matmul(ctx_e[:, st:], lhsT=vts[j][:, 65 * he:65 * he + 65],
                                 rhs=pt[:, st:CH], start=(ji == 0),
                                 stop=(ji == len(js) - 1), skip_group_check=True)
                nc.tensor.matmul(ctx_o[:, st:], lhsT=vts[j][:, 65 * ho:65 * ho + 65],
                                 rhs=pt[:, CH + st:], start=(ji == 0),
                                 stop=(ji == len(js) - 1), skip_group_check=True)
                if ji in (1, 2) and fin_prev:
                    fin_prev[ji]()
                fill(550)

            # build deferred normalize closures for this wave
            state = {}

            def stage1():
                for par, ctx_ps in enumerate((ctx_e, ctx_o)):
                    stg = sgp.tile([65, CH], F32, name="stg", tag="stg")
                    nc.vector.tensor_copy(stg[:], ctx_ps[:])
                    srow = sgp.tile([1, CH], F32, name="srow", tag="srow")
                    nc.vector.tensor_copy(srow[:], stg[64:65, :])
                    rc = sgp.tile([1, CH], F32, name="recip", tag="recip")
                    nc.vector.reciprocal_approx_fast(rc[:], srow[:])
                    state[par] = (stg, rc)

            def stage2():
                for par in (0, 1):
                    stg, rc = state[par]
                    bc = bcp.tile([64, CH], F32, name="bcast", tag="bcast")
                    nc.gpsimd.partition_broadcast(bc[:], rc[:])
                    state[par] = (stg, bc)

            def stage3():
                for par, rows in ((0, slice(0, 64)), (1, slice(64, 128))):
                    stg, bc = state[par]
                    nc.vector.tensor_tensor(cts[g][rows, CH * c:CH * (c + 1)],
                                            stg[0:64, :], bc[:],
                                            op=mybir.AluOpType.mult)
            return [stage1, stage2, stage3], pt

        # --- schedule -----------------------------------------------------
        # startup: direct emission; first wave (0,0) needs only wk/wq/x0/wv
        emit_now("k0_0", qk_unit(kts[0], wk_s, 0, 0))
        emit_now("q0_0", qk_unit(qts[0], wq_s, 0, 0))
        for m in (0, 1, 2, 3):
            emit_now(f"v{m}", v_unit(m))

        # filler queue in consumption order
        QK_COST, V_COST, OP_COST = 1700, 1700, 900
        for g in (1, 2, 3):
            units.append((f"k{g}_0", QK_COST, qk_unit(kts[g], wk_s, g, 0)))
            units.append((f"q{g}_0", QK_COST, qk_unit(qts[g], wq_s, g, 0)))
        for m in (4, 5, 6, 7):
            units.append((f"v{m}", V_COST, v_unit(m)))
        for g in range(4):
            units.append((f"k{g}_1", QK_COST, qk_unit(kts[g], wk_s, g, 1)))
            units.append((f"q{g}_1", QK_COST, qk_unit(qts[g], wq_s, g, 1)))
        for m in (8, 9, 10, 11):
            units.append((f"v{m}", V_COST, v_unit(m)))
        for g in range(4):
            units.append((f"k{g}_2", QK_COST, qk_unit(kts[g], wk_s, g, 2)))
            units.append((f"q{g}_2", QK_COST, qk_unit(qts[g], wq_s, g, 2)))
        for m in (12, 13, 14, 15):
            units.append((f"v{m}", V_COST, v_unit(m)))
        for g in range(4):
            units.append((f"k{g}_3", QK_COST, qk_unit(kts[g], wk_s, g, 3)))
            units.append((f"q{g}_3", QK_COST, qk_unit(qts[g], wq_s, g, 3)))

        # wave order: chunk-0 first (smallest data need -> earliest start),
        # (3,0) held back to the end so the final out-proj (op0) trails a
        # small wave; op(c) units go to the BACK of the queue so their cts
        # reads are waves old by the time they hit the PE queue (a fresh
        # read stalls ~2.6us on the normalize chain).
        WAVES = [(0, 0), (1, 0), (2, 0),
                 (0, 1), (1, 1), (2, 1), (3, 1),
                 (0, 2), (1, 2), (2, 2), (3, 2),
                 (0, 3), (1, 3), (2, 3), (3, 3), (3, 0)]
        done_per_chunk = {c: 0 for c in range(NCH)}
        fin = None
        pending_ops = []
        pending_ops2 = []
        for g, c in WAVES:
            fin, pt_last = wave(g, c, fin)
            # ops for a completed chunk are queued two waves later: their cts
            # reads must trail the deferred normalize by enough that the op
            # LDWEIGHTS never waits on a fresh stage3 mult
            units.extend(pending_ops2)
            pending_ops2 = pending_ops
            pending_ops = []
            fill(1500)
            # HAM keepalives: one fires immediately after the wave's last PV,
            # one fires when the wave's last exp completes (mid-gap), so a
            # boundary stall never shows the PE_HAM a full idle window
            kp = ps_mm.tile([128, CH], F32, name="mm", tag="mm")
            nc.tensor.matmul(kp[:, 0:128], lhsT=wscr[:, 0:128],
                             rhs=wscr[:, 0:128])
            nc.tensor.matmul(kp[:, 128:256], lhsT=wscr[:, 0:128],
                             rhs=pt_last[:, 384:512])
            done_per_chunk[c] += 1
            if done_per_chunk[c] == 4:
                pending_ops = [(f"op{c}_{qb}_{n}", OP_COST, op_unit(qb, n))
                               for qb in range(4 * c, 4 * c + 4) for n in range(2)]

        for s in fin:   # finalize the last wave
            s()
        units.extend(pending_ops2)
        units.extend(pending_ops)
        while units:
            emit_next()

    nc.compile()
    return nc


def _get_module():
    global _CACHED
    if _CACHED is None:
        _CACHED = build_module()
    return _CACHED


def _causal_mask_tile():
    k = np.arange(128)[:, None]
    q = np.arange(128)[None, :]
    return np.where(k <= q, 0.0, NEG).astype(np.float32)


def _build_in_maps(inputs, Wq, Wk, Wv, Wo):
    bf = ml_dtypes.bfloat16
    mask = _causal_mask_tile().astype(bf)
    ident = np.eye(128, dtype=np.float32).astype(bf)
    in_maps = []
    for c in range(NCORES):
        b, hh = c // 2, c % 2
        cols = slice(DH * hh, DH * (hh + 1))
        in_maps.append({
            "xT": np.ascontiguousarray(inputs[b].T).astype(bf),
            "wq": np.ascontiguousarray(Wq[:, cols]).astype(bf),
            "wk": np.ascontiguousarray(Wk[:, cols]).astype(bf),
            "wv": np.ascontiguousarray(Wv[:, cols]).astype(bf),
            "wo": np.ascontiguousarray(Wo[cols, :]).astype(bf),
            "maskt": mask,
            "ident": ident,
        })
    return in_maps


def kernel(inputs, Wq, Wk, Wv, Wo, bo):
    inputs = np.asarray(inputs, dtype=np.float32)
    Wq = np.asarray(Wq, dtype=np.float32)
    Wk = np.asarray(Wk, dtype=np.float32)
    Wv = np.asarray(Wv, dtype=np.float32)
    Wo = np.asarray(Wo, dtype=np.float32)
    bo = np.asarray(bo, dtype=np.float32)

    in_maps = _build_in_maps(inputs, Wq, Wk, Wv, Wo)
    nc = _get_module()
    res = bass_utils.run_bass_kernel_spmd(nc, in_maps, core_ids=list(range(NCORES)))
    outs = [r["out"] for r in res.results]

    full = np.empty((B, S, D), dtype=np.float32)
    for b in range(B):
        full[b] = outs[2 * b] + outs[2 * b + 1] + bo[None, :]
    return full
